# revision 20
# baseline (speedup 1.0000x reference)
"""Causal self-attention (B=2, T=2048, C=1024, NH=16, HD=64) on 8 TRN2 cores.

Sharding: TP over heads x DP over batch. Core i handles batch i//4 and
heads [4*(i%4) .. 4*(i%4)+4). v2 design:

  1. QK^T projection in transposed layout, c_in-outer loops so the
     stationary weight is reused across 2 chunk matmuls; PSUM eviction
     (+bias, ->bf16) on ScalarE (Identity activation with per-partition
     bias) since ScalarE is idle before attention starts.
  2. V projection in natural layout, ones-augmented per head (gives the
     softmax denominators for free in the PV matmul).
  3. Flash-style causal attention per head-pair in S^T=[k,q] layout.
     S matmuls for both heads of a pair drain into ONE 2-bank PSUM tile
     ([h0 512-block | h1 512-block]); a single big ScalarE Exp
     instruction (N=2w, strided src) converts both to bf16 P tiles,
     halving ACTIVATE instruction-overhead vs per-head exp. Causal mask
     on the diagonal 128-block only (DVE). PV accumulates y_aug^T over
     k-tiles; denominators broadcast via a K=1 fp16 matmul; normalize on
     DVE.
  4. Output stage via AllGather: per q-chunk each core ships its y^T
     block [256 d, 512 t] (bf16); every core then holds the full
     y^T [1024, 512] at *identical* (SPMD friendly) offsets and computes
     out^T[, its 256 c_out cols] for ALL t with its w_proj column shard,
     accumulating over the full d=1024 in fp32 PSUM (bias per-partition
     at eviction). The host transposes/concats c-shards. This replaces
     the baseline's partial-sum ReduceScatter: collective bytes drop,
     the reduction happens in PE fp32, and the tail shrinks.
  5. PE kept dense (HAM warm): warmup matmuls during the input DMA,
     V-projection quads + output projections of earlier chunks
     interleaved into the attention stream as filler.

Compute dtype bf16 (fp32 PSUM accumulation everywhere).
"""
import sys
import types

import numpy as np
import ml_dtypes

import concourse.bass as bass
import concourse.bacc as bacc
import concourse.tile as tile
import concourse.mybir as mybir
from concourse.bass_utils import run_bass_kernel_spmd

B, T, C, NH, HD = 2, 2048, 1024, 16, 64
N_CORES = 8
TP, DP = 4, 2
HLOC = NH // TP            # 4 heads per core
DLOC = HLOC * HD           # 256
GROUPS = [[0, 1, 2, 3], [4, 5, 6, 7]]
NKT = T // 128             # 16 k-tiles / t-tiles
NCT = C // 128             # 8 c_in tiles
NQC = T // 512             # 4 q-chunks of 512
ES = 512

F32 = mybir.dt.float32
F16 = mybir.dt.float16
BF16 = mybir.dt.bfloat16
AF = mybir.ActivationFunctionType
BF16_NP = ml_dtypes.bfloat16

_CACHED_NC = None
N_WARMUP = 20


def _install_ntff_hook():
    """Register the axon NTFF profiling shim if the image lacks it."""
    if "antenv.axon_hooks" in sys.modules:
        return
    try:
        from trn_agent_boot.trn_boot import _ntff_profile_via_ctypes
        hook = _ntff_profile_via_ctypes("/opt/axon/libaxon_pjrt.so")
        import antenv
        mod = types.ModuleType("antenv.axon_hooks")
        mod.get_axon_ntff_profile_hook = lambda: hook
        mod.set_axon_ntff_profile_hook = lambda h: None
        sys.modules["antenv.axon_hooks"] = mod
        antenv.axon_hooks = mod
    except Exception:
        pass


def build_kernel_body(nc, tc, es, d):
    sbuf = es.enter_context(tc.tile_pool(name="sbuf", bufs=1))
    sbuf2 = es.enter_context(tc.tile_pool(name="sbuf2", bufs=2))
    ppool = es.enter_context(tc.tile_pool(name="ppool", bufs=4))
    spool = es.enter_context(tc.tile_pool(name="spool", bufs=2, space="PSUM"))
    ypool = es.enter_context(tc.tile_pool(name="ypool", bufs=3, space="PSUM"))
    mpool = es.enter_context(tc.tile_pool(name="mpool", bufs=1, space="PSUM"))

    # ---- constants ---------------------------------------------------------
    ones1 = sbuf.tile([1, 128], BF16, tag="ones1")
    nc.vector.memset(ones1[:], 1.0)
    ones16 = sbuf.tile([1, 64], F16, tag="ones16")
    nc.vector.memset(ones16[:], 1.0)
    wrow = sbuf.tile([1, 512], BF16, tag="wrow")
    nc.vector.memset(wrow[:], 0.5)

    # ---- PE warmup during input DMA (keeps HAM at 8/8 from the start) ------
    wps = mpool.tile([128, 512], F32, tag="mm", name="warm")
    for i in range(N_WARMUP):
        nc.tensor.matmul(wps[:], ones1[:], wrow[:], start=True, stop=True)

    # ---- input DMAs, spread across the three DGE queues --------------------
    # sync: small consts, then per-c (wqk + xT first half) so the QK c-loop
    # can start as soon as the early tiles land.
    mask = sbuf.tile([128, 128], BF16, tag="mask")
    nc.sync.dma_start(mask[:], d["mask"][:])
    bqk = [sbuf.tile([128, 1], F32, tag=f"bqk{m}", name=f"bqk{m}") for m in range(4)]
    for m in range(4):
        nc.sync.dma_start(bqk[m][:], d["b_qk"][m * 128:(m + 1) * 128, :])
    bv = sbuf.tile([1, DLOC], BF16, tag="bv")
    nc.sync.dma_start(bv[:], d["b_v"][:])
    bpt = [sbuf.tile([128, 1], F32, tag=f"bp{j}", name=f"bp{j}") for j in range(2)]
    for j in range(2):
        nc.sync.dma_start(bpt[j][:], d["b_p"][j * 128:(j + 1) * 128, :])

    xT = [sbuf.tile([128, T], BF16, tag=f"xT{c}", name=f"xT{c}") for c in range(NCT)]
    wqk = [sbuf.tile([128, 512], BF16, tag=f"wqk{c}", name=f"wqk{c}") for c in range(NCT)]
    for c in range(NCT):
        nc.sync.dma_start(wqk[c][:], d["w_qk"][c * 128:(c + 1) * 128, :])
        if c < 4:
            nc.sync.dma_start(xT[c][:, 0:1024],
                              d["xT"][c * 128:(c + 1) * 128, 0:1024])
    for c in range(4, NCT):
        nc.scalar.dma_start(xT[c][:, 0:1024],
                            d["xT"][c * 128:(c + 1) * 128, 0:1024])
    for c in range(NCT):
        nc.scalar.dma_start(xT[c][:, 1024:2048],
                            d["xT"][c * 128:(c + 1) * 128, 1024:2048])
    # gpsimd queue: V weights + projection weight shard
    wv = [sbuf.tile([128, DLOC], BF16, tag=f"wv{c}", name=f"wv{c}") for c in range(NCT)]
    for c in range(NCT):
        nc.gpsimd.dma_start(wv[c][:], d["w_v"][c * 128:(c + 1) * 128, :])
    wp = [sbuf.tile([128, DLOC], BF16, tag=f"wp{k}", name=f"wp{k}")
          for k in range(NCT)]
    for k in range(NCT):
        nc.gpsimd.dma_start(wp[k][:], d["w_proj"][k * 128:(k + 1) * 128, :])

    # ---- QK^T projection (transposed layout, c-outer for LDW reuse) --------
    # qkT[m]: [128, T] bf16; m=0: Q^T heads 0,1  m=1: K^T heads 0,1
    #                        m=2: Q^T heads 2,3  m=3: K^T heads 2,3
    qkT = [sbuf.tile([128, T], BF16, tag=f"qkT{m}", name=f"qkT{m}") for m in range(4)]
    wcol_of_m = [0, 2, 1, 3]  # m-tile -> w_qk column block [q01|q23|k01|k23]

    def emit_qk(m):
        wc = wcol_of_m[m]
        for half in range(2):
            ps = spool.tile([128, 1024], F32, tag="S", name="qkps")
            for c in range(NCT):
                for sub in range(2):
                    nc.tensor.matmul(
                        ps[:, sub * 512:(sub + 1) * 512],
                        wqk[c][:, wc * 128:(wc + 1) * 128],
                        xT[c][:, half * 1024 + sub * 512:half * 1024 + (sub + 1) * 512],
                        start=(c == 0), stop=(c == NCT - 1))
            for sub in range(2):
                off = half * 1024 + sub * 512
                nc.scalar.activation(
                    qkT[m][:, off:off + 512], ps[:, sub * 512:(sub + 1) * 512],
                    AF.Identity, bias=bqk[wc][:], scale=1.0)

    # ---- V projection (natural layout, ones-augmented), quads of 4 t-tiles -
    vsb = [sbuf.tile([128, HLOC * (HD + 1)], BF16, tag=f"vsb{tt}", name=f"vsb{tt}")
           for tt in range(NKT)]

    def emit_v_quad(q):
        ps = spool.tile([128, 1024], F32, tag="S", name="vps")
        for j in range(4):
            tt = 4 * q + j
            for c in range(NCT):
                nc.tensor.matmul(
                    ps[:, j * 256:(j + 1) * 256],
                    xT[c][:, tt * 128:(tt + 1) * 128], wv[c][:],
                    start=(c == 0), stop=False)
            nc.tensor.matmul(ps[:, j * 256:(j + 1) * 256], ones1[:], bv[:],
                             start=False, stop=True)
        for j in range(4):
            tt = 4 * q + j
            vgrp = vsb[tt][:].rearrange("p (h x) -> p h x", h=HLOC)
            nc.scalar.activation(
                vgrp[:, :, 0:HD],
                ps[:, j * 256:(j + 1) * 256].rearrange("p (h x) -> p h x", h=HLOC),
                AF.Copy)
            nc.vector.memset(vgrp[:, :, HD:HD + 1], 1.0)

    # ---- attention inner loop ---------------------------------------------
    yn = [sbuf.tile([128, T], BF16, tag=f"yn{p}", name=f"yn{p}") for p in range(2)]

    def emit_att(qc, p):
        qt, kt = qkT[2 * p], qkT[2 * p + 1]
        nki = 4 * qc + 4
        yps = [ypool.tile([65, 512], F32, tag="y", name=f"yps{qc}{p}{h}")
               for h in range(2)]
        pts = {}

        def width(ki):
            return ES - max(0, 128 * ki - ES * qc)

        def emit_pv(ki):
            w = width(ki)
            pt = pts.pop(ki)
            for h in range(2):
                head = 2 * p + h
                nc.tensor.matmul(
                    yps[h][:, ES - w:ES],
                    vsb[ki][:, head * 65:head * 65 + 65],
                    pt[:, h * w:h * w + w],
                    start=(ki == 0), stop=(ki == nki - 1))

        for ki in range(nki):
            w = width(ki)
            qs = max(ES * qc, 128 * ki)
            s = spool.tile([128, 1024], F32, tag="S", name="satt")
            for h in range(2):
                nc.tensor.matmul(
                    s[:, 512 * h:512 * h + w],
                    kt[64 * h:64 * h + 64, ki * 128:(ki + 1) * 128],
                    qt[64 * h:64 * h + 64, qs:qs + w],
                    start=True, stop=True, tile_position=(64 * h, 0))
            pt = ppool.tile([128, 1024], BF16, tag="P", name="pt")
            pts[ki] = pt
            nc.scalar.activation(
                pt[:, 0:2 * w].rearrange("p (b x) -> p b x", b=2),
                s[:].rearrange("p (b x) -> p b x", b=2)[:, :, 0:w],
                AF.Exp, scale=0.125)
            if 128 * ki >= ES * qc:
                for h in range(2):
                    nc.vector.tensor_mul(
                        pt[:, h * w:h * w + 128], pt[:, h * w:h * w + 128], mask[:])
            if ki >= 2:
                emit_pv(ki - 2)
        emit_pv(nki - 2)
        emit_pv(nki - 1)
        for h in range(2):
            srow = sbuf2.tile([1, ES], F32, tag="srow", name="srow")
            nc.vector.tensor_copy(srow[:], yps[h][64:65, :])
            rec = sbuf2.tile([1, ES], F32, tag="rec", name="rec")
            nc.vector.reciprocal_approx_fast(rec[:], srow[:])
            rec16 = sbuf2.tile([1, ES], F16, tag="rec16", name="rec16")
            nc.vector.tensor_copy(rec16[:], rec[:])
            bcps = mpool.tile([128, ES], F32, tag="mm", name="bcps")
            nc.tensor.matmul(bcps[0:64, :], ones16[:], rec16[:],
                             start=True, stop=True)
            bc = sbuf2.tile([64, ES], F32, tag="bc", name="bc")
            nc.vector.tensor_copy(bc[:], bcps[0:64, :])
            nc.vector.tensor_mul(
                yn[p][64 * h:64 * h + 64, qc * ES:(qc + 1) * ES],
                yps[h][0:64, :], bc[:])

    # ---- output stage: AllGather y^T chunk + c_out-sharded projection ------
    def emit_yndp(qc, p):
        nc.sync.dma_start(
            d["ynd"][256 * qc + 128 * p:256 * qc + 128 * p + 128, :],
            yn[p][:, qc * ES:(qc + 1) * ES])

    def emit_ag(qlo, qhi):
        nc.gpsimd.collective_compute(
            "AllGather",
            mybir.AluOpType.bypass,
            replica_groups=GROUPS,
            ins=[d["ynd"][256 * qlo:256 * (qhi + 1), :].opt()],
            outs=[d["ag"][1024 * qlo:1024 * (qhi + 1), :].opt()],
        )

    def emit_proj(qc, merged=False):
        # yt col-block k = y^T rows [128k:128k+128] of this q-chunk.
        # Solo AG (one qc): ag rows are [1024*qc + 128k].  Merged AG over
        # qc 0..1: AllGather concatenates by RANK, so rank g's two d-blocks
        # for chunk qc sit at rows [512g + 256*qc + 128*(k%2)], g = k//2.
        yt = sbuf2.tile([128, 8 * ES], BF16, tag="yt", name=f"yt{qc}")
        for k in range(NCT):
            if merged:
                row = 512 * (k // 2) + 256 * qc + 128 * (k % 2)
            else:
                row = 1024 * qc + 128 * k
            eng = nc.sync if k % 2 == 0 else nc.scalar
            eng.dma_start(
                yt[:, k * ES:(k + 1) * ES],
                d["ag"][row:row + 128, :])
        # out^T[cc*128 : +128, qc chunk] = sum_k wp[k][:,cc].T @ yT[k]
        pout = sbuf2.tile([128, 2 * ES], F16, tag="pout", name=f"pout{qc}")
        for cc in range(2):
            ps = mpool.tile([128, ES], F32, tag="mm", name="projps")
            for k in range(NCT):
                nc.tensor.matmul(
                    ps[:], wp[k][:, cc * 128:(cc + 1) * 128],
                    yt[:, k * ES:(k + 1) * ES],
                    start=(k == 0), stop=(k == NCT - 1))
            nc.vector.tensor_scalar_add(
                pout[:, cc * ES:(cc + 1) * ES], ps[:], bpt[cc][:])
            nc.sync.dma_start(
                d["out"][cc * 128:(cc + 1) * 128, qc * ES:(qc + 1) * ES],
                pout[:, cc * ES:(cc + 1) * ES])

    # ---- schedule ----------------------------------------------------------
    # q-chunks processed DESCENDING so the final AG+proj chain hangs off the
    # smallest chunk; each proj is emitted a full attention block after its
    # AllGather trigger so the (ring, ~15-25us) collective is hidden.
    emit_qk(0)
    emit_qk(1)
    emit_qk(2)
    emit_qk(3)
    for q in range(4):
        emit_v_quad(q)
    emit_att(3, 0)
    emit_yndp(3, 0)
    emit_att(3, 1)
    emit_yndp(3, 1)
    emit_ag(3, 3)
    emit_att(2, 0)
    emit_yndp(2, 0)
    emit_att(2, 1)
    emit_yndp(2, 1)
    emit_ag(2, 2)
    emit_att(1, 0)
    emit_yndp(1, 0)
    emit_proj(3)
    emit_att(1, 1)
    emit_yndp(1, 1)
    emit_att(0, 0)
    emit_yndp(0, 0)
    emit_proj(2)
    emit_att(0, 1)
    emit_yndp(0, 1)
    emit_ag(0, 1)
    emit_proj(1, merged=True)
    emit_proj(0, merged=True)


def build_nc():
    global _CACHED_NC
    if _CACHED_NC is not None:
        return _CACHED_NC
    nc = bacc.Bacc("TRN2", target_bir_lowering=False, debug=False,
                   num_devices=N_CORES)
    d = {
        "xT": nc.dram_tensor("xT", [C, T], BF16, kind="ExternalInput").ap(),
        "w_qk": nc.dram_tensor("w_qk", [C, 2 * DLOC], BF16,
                               kind="ExternalInput").ap(),
        "b_qk": nc.dram_tensor("b_qk", [2 * DLOC, 1], F32,
                               kind="ExternalInput").ap(),
        "w_v": nc.dram_tensor("w_v", [C, DLOC], BF16,
                              kind="ExternalInput").ap(),
        "b_v": nc.dram_tensor("b_v", [1, DLOC], BF16,
                              kind="ExternalInput").ap(),
        "w_proj": nc.dram_tensor("w_proj", [C, DLOC], BF16,
                                 kind="ExternalInput").ap(),
        "b_p": nc.dram_tensor("b_p", [DLOC, 1], F32,
                              kind="ExternalInput").ap(),
        "mask": nc.dram_tensor("mask", [128, 128], BF16,
                               kind="ExternalInput").ap(),
        "out": nc.dram_tensor("out", [DLOC, T], F16,
                              kind="ExternalOutput").ap(),
        "ynd": nc.dram_tensor("ynd", [4 * DLOC, ES], BF16).ap(),
        "ag": nc.dram_tensor("ag", [4 * C, ES], BF16).ap(),
    }
    from contextlib import ExitStack
    with tile.TileContext(nc) as tc, ExitStack() as es:
        build_kernel_body(nc, tc, es, d)
    nc.compile()
    _CACHED_NC = nc
    return nc


def make_in_maps(x, w_attn, b_attn, w_proj, b_proj):
    x = np.asarray(x, dtype=np.float32)
    w_attn = np.asarray(w_attn, dtype=np.float32)
    b_attn = np.asarray(b_attn, dtype=np.float32)
    w_proj = np.asarray(w_proj, dtype=np.float32)
    b_proj = np.asarray(b_proj, dtype=np.float32)

    # causal mask for the S^T-layout diagonal block: valid iff q >= k
    kr = np.arange(128)
    mask = (kr[None, :] >= kr[:, None]).astype(BF16_NP)  # [k,q]

    in_maps = []
    for i in range(N_CORES):
        b = i // TP
        g = i % TP
        heads = list(range(HLOC * g, HLOC * g + HLOC))
        qcols = np.concatenate(
            [np.arange(h * HD, (h + 1) * HD) for h in heads])
        kcols = qcols + C
        vcols = qcols + 2 * C
        # w_qk column blocks: [q01 | q23 | k01 | k23] (128 cols each)
        w_qk = np.concatenate(
            [w_attn[:, qcols], w_attn[:, kcols]], axis=1)
        b_qk = np.concatenate([b_attn[qcols], b_attn[kcols]])
        xT = np.ascontiguousarray(x[b].T)
        in_maps.append({
            "xT": xT.astype(BF16_NP),
            "w_qk": w_qk.astype(BF16_NP),
            "b_qk": b_qk[:, None].astype(np.float32),
            "w_v": w_attn[:, vcols].astype(BF16_NP),
            "b_v": b_attn[vcols][None, :].astype(BF16_NP),
            "w_proj": w_proj[:, DLOC * g:DLOC * (g + 1)].astype(BF16_NP),
            "b_p": b_proj[DLOC * g:DLOC * (g + 1)][:, None].astype(np.float32),
            "mask": mask,
        })
    return in_maps


def run(x, w_attn, b_attn, w_proj, b_proj, trace=False):
    _install_ntff_hook()
    nc = build_nc()
    in_maps = make_in_maps(x, w_attn, b_attn, w_proj, b_proj)
    res = run_bass_kernel_spmd(nc, in_maps, list(range(N_CORES)), trace=trace)
    out = np.empty((B, T, C), dtype=np.float32)
    for i in range(N_CORES):
        b = i // TP
        g = i % TP
        o = res.results[i]["out"].astype(np.float32)  # [256 c, 2048 t]
        out[b, :, DLOC * g:DLOC * (g + 1)] = o.T
    return out, res


def kernel(x, w_attn, b_attn, w_proj, b_proj):
    out, _ = run(x, w_attn, b_attn, w_proj, b_proj, trace=False)
    return out
